# revision 2
# baseline (speedup 1.0000x reference)
"""Committee-vote histogram kernel for TRN2 (8 NeuronCores, data-parallel).

votes[b, c] = sum_m 1[argmax_c' (x[b] @ W[m, :, c'] + b[m, c']) == c]

Strategy per core (batch shard of 8192 rows):
  - x is decomposed host-side into an exact fp16 pair (x = xh + xl with
    residual ~2^-22|x|); likewise W. Logits are computed as
    xh@Wh + xh@Wl + xl@Wh (+bias), whose decomposition error (~2e-7) is at
    fp32 rounding level - validated exact-match against the fp32 reference.
  - The host packs each core's x halves into ONE array [128, 4*8192] fp16
    whose rows mirror the SBUF chunk tiles exactly (per chunk: [k, h, b]
    blocks), so every chunk DMA is 128 contiguous multi-KB descriptors.
    k=0 halves stream on the sync HWDGE queue, k=1 on the scalar queue, so
    the PE's k-phased pipeline starts after the first 0.25 MB lands.
  - While the first chunk is in flight the PE runs warm-up matmuls on a
    scratch PSUM tile so the p-state clock ramp starts before real work.
  - PSUM is pre-seeded with the f32 bias by the Scalar (ACT) engine
    (copy SBUF->PSUM), freeing the PE of all bias matmuls.
  - The two xh passes are FUSED into one matmul per (tile, k): rhs is the
    concatenated (wh|wl) 160 columns and the out AP carries a stride-0
    dim that folds columns 80..159 back onto 0..79, so both products
    accumulate into the same PSUM cells (start=False => every column
    write accumulates).
  - W columns are packed c-major (col = c*8 + m), so the vote stage is 3
    DVE ops per 8-tile super-batch: reduce_max over c (strided view),
    is_ge into an m-innermost fp16 mask, and one unit-stride fp16
    tensor_reduce(add) over m straight into the staging tile (counts <= 8
    are exact in fp16). Each super-batch's votes are stored with their own
    small DMA so only the last ~10KB store sits on the tail.
  - The host de-interleaves [p, sb, g, t, c] -> [b, c] and casts to f32.
"""

import os
import sys

import numpy as np

if os.path.isdir("/opt/trn_rl_repo") and "/opt/trn_rl_repo" not in sys.path:
    sys.path.insert(0, "/opt/trn_rl_repo")

import concourse.bass as bass
import concourse.tile as tile
from concourse import bacc, mybir

F32 = mybir.dt.float32
F16 = mybir.dt.float16

B_FULL = 65536
D = 256
C = 10
M = 8
N_CORES = 8
B_SHARD = B_FULL // N_CORES  # 8192
P = 128

MC = M * C  # 80 logit columns per sample
CHUNKS = (512, 1536, 2048, 2048, 1536, 512)  # rows per input-DMA chunk
WARMUP_MMS = 10


def build_nc(b_shard: int = B_SHARD) -> bass.Bass:
    assert sum(CHUNKS) == b_shard
    n_tiles = b_shard // P  # 64
    n_sb = n_tiles // 8  # 8 super-batches of 8 tiles (2 PSUM banks x 4)

    nc = bacc.Bacc("TRN2", target_bir_lowering=False)
    # packed x halves: per chunk c with L rows at column base, cols
    # [4*base + (k*2 + h)*L + 0..L) hold half (k, h) in [d-in-k, b] layout
    xin = nc.dram_tensor("xin", [P, 4 * b_shard], F16, kind="ExternalInput")
    whl = nc.dram_tensor("whl", [D, 2 * MC], F16, kind="ExternalInput")
    brep = nc.dram_tensor("brep", [P, 8 * MC], F32, kind="ExternalInput")
    # votes in SBUF staging layout [p, sb*80 + g*40 + t*10 + c]
    y = nc.dram_tensor("y", [P, n_tiles * C], F16, kind="ExternalOutput")

    with tile.TileContext(nc) as tc:
        with (
            tc.tile_pool(name="consts", bufs=1) as consts,
            tc.tile_pool(name="xt", bufs=len(CHUNKS)) as xt_pool,
            tc.tile_pool(name="warm", bufs=1, space="PSUM") as warm_pool,
            tc.tile_pool(name="lg", bufs=3, space="PSUM") as lg_pool,
            tc.tile_pool(name="mx", bufs=3) as mx_pool,
            tc.tile_pool(name="eq", bufs=3) as eq_pool,
            tc.tile_pool(name="stg", bufs=1) as stg_pool,
        ):
            # --- sync-queue DMAs: whl const then every chunk's k=0 half ---
            whl_sb = consts.tile([P, 2, 2, MC], F16)
            nc.sync.dma_start(
                whl_sb, whl.rearrange("(k p) (h c) -> p k h c", p=P, h=2)
            )
            xts = []
            base = 0
            for L in CHUNKS:
                xt = xt_pool.tile([P, 2, 2, L], F16, name="xt")
                xts.append(xt)
                nc.sync.dma_start(
                    xt[:, 0],
                    xin[:, 4 * base : 4 * base + 2 * L].rearrange(
                        "p (h l) -> p h l", h=2
                    ),
                )
                base += L

            # --- scalar-queue DMAs (k=1 halves) + ACT bias pre-seeding ---
            # lg tiles are created up front so the bias copies for the first
            # super-batches can be interleaved between the DMA issues; the
            # copies for SB>=3 stall on PSUM buf reuse, so they go last.
            brep_sb = consts.tile([P, 2, 4 * MC], F32)
            nc.scalar.dma_start(
                brep_sb, brep.rearrange("p (g c) -> p g c", g=2)
            )
            lgs = [
                lg_pool.tile([P, 2, 512], F32, name="lg") for _ in range(n_sb)
            ]

            def seed_bias(sb):
                nc.scalar.copy(lgs[sb][:, :, : 4 * MC], brep_sb)

            base = 0
            for ci, L in enumerate(CHUNKS):
                nc.scalar.dma_start(
                    xts[ci][:, 1],
                    xin[:, 4 * base + 2 * L : 4 * base + 4 * L].rearrange(
                        "p (h l) -> p h l", h=2
                    ),
                )
                if ci < 3:
                    seed_bias(ci)
                base += L
            for sb in range(3, n_sb):
                seed_bias(sb)

            # --- PE warm-up while the first chunk is in flight ---
            ones_g = consts.tile([P, 512], F16)
            nc.gpsimd.memset(ones_g, 1.0)
            warm = warm_pool.tile([P, 512], F32)
            for _ in range(WARMUP_MMS):
                nc.tensor.matmul(
                    warm, lhsT=ones_g[:, :P], rhs=ones_g, start=True, stop=True
                )

            stg = stg_pool.tile([P, n_tiles * C], F16)

            # global tile T -> (chunk index, within-chunk column)
            tile_map = []
            base = 0
            for ci, L in enumerate(CHUNKS):
                for t in range(L // P):
                    tile_map.append((ci, t * P))
                base += L

            # --- main pipeline: super-batches of 8 tiles (may span chunks) ---
            for SB in range(n_sb):
                lg = lgs[SB]
                # k-phased so phase 0 only needs the k=0 x halves
                for k in range(2):
                    for j in range(8):
                        g, o = j // 4, (j % 4) * MC
                        ci, col = tile_map[SB * 8 + j]
                        xt = xts[ci]
                        xh_c = xt[:, k, 0, col : col + P]
                        xl_c = xt[:, k, 1, col : col + P]
                        out = lg[:, g, o : o + MC]
                        last = k == 1 and (j % 4) == 3
                        # xh@wh + xh@wl in ONE matmul: the out AP's
                        # stride-0 h dim folds columns 80..159 onto
                        # 0..79, accumulating both products (start=False
                        # means every column-write accumulates)
                        nc.tensor.matmul(
                            out[:, None, :].broadcast_to([P, 2, MC]),
                            lhsT=xh_c, rhs=whl_sb[:, k],
                            start=False, stop=False,
                        )
                        nc.tensor.matmul(
                            out, lhsT=xl_c, rhs=whl_sb[:, k, 0, :],
                            start=False, stop=last,
                        )

                # votes: 3 DVE ops over both banks, except the final
                # super-batch where per-bank chains shorten the tail latency
                for gs, ge in ((0, 2),) if SB < n_sb - 1 else ((0, 1), (1, 2)):
                    ng = ge - gs
                    lg_g = lg[:, gs:ge, : 4 * MC]
                    # view with c strided / m innermost (W packed c-major)
                    lgv_m = lg_g.rearrange("p g (t c m) -> p g t m c", c=C, m=M)
                    lgv_c = lg_g.rearrange("p g (t c m) -> p g t c m", c=C, m=M)
                    mx = mx_pool.tile([P, 2, 4, M], F32, name="mx")
                    mxv = mx[:, gs:ge]
                    nc.vector.reduce_max(mxv, lgv_m, axis=mybir.AxisListType.X)
                    # mask in fp16, unit-stride (g, t, c, m) write
                    eq = eq_pool.tile([P, 2, 4, C, M], F16, name="eq")
                    eqv = eq[:, gs:ge]
                    nc.vector.tensor_tensor(
                        out=eqv,
                        in0=lgv_c,
                        in1=mxv[:, :, :, None, :].broadcast_to(
                            [P, ng, 4, C, M]
                        ),
                        op=mybir.AluOpType.is_ge,
                    )
                    # member-sum: one unit-stride fp16 reduce over m
                    # (counts <= 8 are exact in fp16)
                    out_v = stg[
                        :, SB * 8 * C + gs * 4 * C :][:, : ng * 4 * C
                    ].rearrange("p (g t c) -> p g t c", g=ng, c=C)
                    with nc.allow_low_precision("vote counts <= 8 exact fp16"):
                        nc.vector.tensor_reduce(
                            out=out_v, in_=eqv,
                            axis=mybir.AxisListType.X,
                            op=mybir.AluOpType.add,
                        )
                    # per-super-batch store so only ~10KB sits on the tail
                    nc.sync.dma_start(
                        y[:, SB * 8 * C + gs * 4 * C :][:, : ng * 4 * C],
                        stg[:, SB * 8 * C + gs * 4 * C :][:, : ng * 4 * C],
                    )
    nc.compile()
    return nc


_NC_CACHE: dict[int, bass.Bass] = {}


def _get_nc(b_shard: int) -> bass.Bass:
    if b_shard not in _NC_CACHE:
        _NC_CACHE[b_shard] = build_nc(b_shard)
    return _NC_CACHE[b_shard]


def make_in_maps(x: np.ndarray, W: np.ndarray, b: np.ndarray):
    """Host-side prep: exact fp16 pair decomposition + per-core packing."""
    xf = np.asarray(x, dtype=np.float32)
    xh = xf.astype(np.float16)
    xl = (xf - xh.astype(np.float32)).astype(np.float16)
    # c-major columns: col index = 8*c + m; wh|wl concatenated per row
    wf = (
        np.asarray(W, dtype=np.float32).transpose(1, 2, 0).reshape(D, MC)
    )
    whf = wf.astype(np.float16)
    wlf = (wf - whf.astype(np.float32)).astype(np.float16)
    whlf = np.ascontiguousarray(np.concatenate([whf, wlf], axis=1))
    bv = np.asarray(b, dtype=np.float32).T.reshape(MC)  # bv[8c+m] = b[m,c]
    brep = np.ascontiguousarray(np.tile(bv, (P, 8)))

    xins = np.empty((N_CORES, P, 4 * B_SHARD), dtype=np.float16)
    halves = (xh, xl)
    for i in range(N_CORES):
        r0 = i * B_SHARD
        base = 0
        for L in CHUNKS:
            for k in range(2):
                for h in range(2):
                    c0 = 4 * base + (k * 2 + h) * L
                    xins[i, :, c0 : c0 + L] = halves[h][
                        r0 + base : r0 + base + L, k * P : (k + 1) * P
                    ].T
            base += L
    return [
        {"xin": xins[i], "whl": whlf, "brep": brep} for i in range(N_CORES)
    ]


def _postprocess(y_raw: np.ndarray) -> np.ndarray:
    # [p, (sb g t) * 10] fp16 -> [tile*128, 10] f32 (small ints: exact)
    n_tiles = y_raw.shape[1] // C
    return (
        y_raw.reshape(P, n_tiles, C)
        .transpose(1, 0, 2)
        .reshape(n_tiles * P, C)
        .astype(np.float32)
    )


def kernel(x: np.ndarray, W: np.ndarray, b: np.ndarray, **_) -> np.ndarray:
    from concourse.bass_utils import run_bass_kernel_spmd

    assert x.shape == (B_FULL, D), x.shape
    in_maps = make_in_maps(x, W, b)
    nc = _get_nc(B_SHARD)
    res = run_bass_kernel_spmd(nc, in_maps, core_ids=list(range(N_CORES)))
    return np.concatenate(
        [_postprocess(res.results[i]["y"]) for i in range(N_CORES)], axis=0
    )


# revision 11
# speedup vs baseline: 1.1149x; 1.1149x over previous
"""Committee-vote histogram kernel for TRN2 (8 NeuronCores, data-parallel).

votes[b, c] = sum_m 1[argmax_c' (x[b] @ W[m, :, c'] + b[m, c']) == c]

Strategy per core (batch shard of 8192 rows):
  - x is decomposed host-side into an exact fp16 pair (x = xh + xl with
    residual ~2^-22|x|); likewise W. Logits are computed as
    xh@Wh + xh@Wl + xl@Wh (+bias), whose decomposition error (~2e-7) is at
    fp32 rounding level - validated exact-match against the fp32 reference.
  - The host packs each core's x halves into ONE array [128, 4*8192] fp16
    whose rows mirror the SBUF chunk tiles exactly (per chunk: [k, h, b]
    blocks), so every chunk DMA is 128 contiguous multi-KB descriptors.
    k=0 halves stream on the sync HWDGE queue, k=1 on the scalar queue, so
    the PE's k-phased pipeline starts after the first 0.25 MB lands.
  - While the first chunk is in flight the PE runs warm-up matmuls on a
    scratch PSUM tile so the p-state clock ramp starts before real work.
  - Logits accumulate per 4-tile GROUP into single-bank PSUM tiles (7 in
    flight) pre-seeded with the f32 bias by the Scalar (ACT) engine,
    freeing the PE of all bias matmuls and keeping PSUM turnover fine-
    grained so the PE never stalls on bank reuse.
  - The two xh passes are FUSED into one matmul per (tile, k): rhs is the
    concatenated (wh|wl) 160 columns and the out AP carries a stride-0
    dim that folds columns 80..159 back onto 0..79, so both products
    accumulate into the same PSUM cells (start=False => every column
    write accumulates).
  - Votes: per group, DVE does reduce_max over c then an is_ge mask into
    fp16 (the second op frees the PSUM bank); the member-sum add-tree
    (exact in fp16 for counts <= 8) runs on the otherwise-idle GpSimd
    engine per super-batch, straight into the staging tile. Each super-
    batch's votes are stored with their own small DMA so only the last
    ~10KB store sits on the tail; the final super-batch runs per-group
    DVE-only chains to shorten the tail.
  - The host de-interleaves [p, sb, g, t, c] -> [b, c] and casts to f32.
"""

import os
import sys

import numpy as np

if os.path.isdir("/opt/trn_rl_repo") and "/opt/trn_rl_repo" not in sys.path:
    sys.path.insert(0, "/opt/trn_rl_repo")

import concourse.bass as bass
import concourse.tile as tile
from concourse import bacc, mybir

F32 = mybir.dt.float32
F16 = mybir.dt.float16

B_FULL = 65536
D = 256
C = 10
M = 8
N_CORES = 8
B_SHARD = B_FULL // N_CORES  # 8192
P = 128

MC = M * C  # 80 logit columns per sample
CHUNKS = (512, 1536, 2048, 2048, 1536, 512)  # rows per input-DMA chunk
WARMUP_MMS = 5


def build_nc(b_shard: int = B_SHARD) -> bass.Bass:
    assert sum(CHUNKS) == b_shard
    n_tiles = b_shard // P  # 64
    n_sb = n_tiles // 8  # 8 super-batches of 8 tiles (2 groups x 4 tiles)
    n_gr = n_tiles // 4  # 16 four-tile groups (one PSUM bank each)

    nc = bacc.Bacc("TRN2", target_bir_lowering=False)
    # packed x halves: per chunk c with L rows at column base, cols
    # [4*base + (k*2 + h)*L + 0..L) hold half (k, h) in [d-in-k, b] layout
    xin = nc.dram_tensor("xin", [P, 4 * b_shard], F16, kind="ExternalInput")
    whl = nc.dram_tensor("whl", [D, 2 * MC], F16, kind="ExternalInput")
    bc2 = nc.dram_tensor("bc2", [2, 4 * MC], F16, kind="ExternalInput")
    # votes in SBUF staging layout [p, sb*80 + g*40 + t*10 + c]
    y = nc.dram_tensor("y", [P, n_tiles * C], F16, kind="ExternalOutput")

    with tile.TileContext(nc) as tc:
        with (
            tc.tile_pool(name="consts", bufs=1) as consts,
            tc.tile_pool(name="xt", bufs=len(CHUNKS)) as xt_pool,
            tc.tile_pool(name="warm", bufs=1, space="PSUM") as warm_pool,
            tc.tile_pool(name="lg", bufs=7, space="PSUM") as lg_pool,
            tc.tile_pool(name="mx", bufs=4) as mx_pool,
            tc.tile_pool(name="eq", bufs=3) as eq_pool,
            tc.tile_pool(name="tsum", bufs=2) as tsum_pool,
            tc.tile_pool(name="stg", bufs=1) as stg_pool,
        ):
            # --- sync-queue DMAs: consts then every chunk's k=0 half ---
            bc2_sb = consts.tile([2, 4 * MC], F16)
            nc.sync.dma_start(bc2_sb, bc2[:])
            whl_sb = consts.tile([P, 2, 2, MC], F16)
            nc.sync.dma_start(
                whl_sb, whl.rearrange("(k p) (h c) -> p k h c", p=P, h=2)
            )
            xts = []
            base = 0
            for L in CHUNKS:
                xt = xt_pool.tile([P, 2, 2, L], F16, name="xt")
                xts.append(xt)
                nc.sync.dma_start(
                    xt[:, 0],
                    xin[:, 4 * base : 4 * base + 2 * L].rearrange(
                        "p (h l) -> p h l", h=2
                    ),
                )
                base += L

            # --- scalar-queue DMAs (k=1 halves) ---
            base = 0
            for ci, L in enumerate(CHUNKS):
                nc.scalar.dma_start(
                    xts[ci][:, 1],
                    xin[:, 4 * base + 2 * L : 4 * base + 4 * L].rearrange(
                        "p (h l) -> p h l", h=2
                    ),
                )
                base += L

            lgs = [
                lg_pool.tile([P, 512], F32, name="lg") for _ in range(n_gr)
            ]
            ones_g = consts.tile([P, 512], F16)
            nc.gpsimd.memset(ones_g, 1.0)

            def seed_bias(gr):
                # seed the bank with the bias: every row of ones.T @ (bh4|
                # bl4) is bh4+bl4, summed exactly in f32 PSUM
                nc.tensor.matmul(
                    lgs[gr][:, : 4 * MC], lhsT=ones_g[:2, :P], rhs=bc2_sb,
                    start=True, stop=False,
                )

            # --- PE warm-up while the first chunk is in flight; then the
            # first 7 PSUM banks are bias-seeded during the same idle window
            # (banks 7+ would stall on reuse, so they seed in the main loop)
            warm = warm_pool.tile([P, 512], F32)
            for _ in range(WARMUP_MMS):
                nc.tensor.matmul(
                    warm, lhsT=ones_g[:, :P], rhs=ones_g, start=True, stop=True
                )
            for gr in range(7):
                seed_bias(gr)

            stg = stg_pool.tile([P, n_tiles * C], F16)

            # global tile T -> (chunk index, within-chunk column)
            tile_map = []
            for ci, L in enumerate(CHUNKS):
                for t in range(L // P):
                    tile_map.append((ci, t * P))

            def vote_mask(gr):
                """reduce_max + is_ge for one group; frees its PSUM bank.
                Returns the eq mask slice [P, 4, M, C] (fp16)."""
                lg = lgs[gr]
                lgv = lg[:, : 4 * MC].rearrange("p (t m c) -> p t m c", m=M, c=C)
                mx = mx_pool.tile([P, 4, M], F32, name="mx")
                nc.vector.reduce_max(mx, lgv, axis=mybir.AxisListType.X)
                eq = eqs[gr // 2][:, gr % 2]
                nc.vector.tensor_tensor(
                    out=eq,
                    in0=lgv,
                    in1=mx[:, :, :, None].broadcast_to([P, 4, M, C]),
                    op=mybir.AluOpType.is_ge,
                )
                return eq

            # --- main pipeline: super-batches of 8 tiles (2 groups) ---
            eqs = []
            for SB in range(n_sb):
                lgA, lgB = lgs[2 * SB], lgs[2 * SB + 1]
                eqs.append(eq_pool.tile([P, 2, 4, M, C], F16, name="eq"))
                for gr in (2 * SB, 2 * SB + 1):
                    if gr >= 7:
                        seed_bias(gr)
                # k-phased so phase 0 only needs the k=0 x halves
                for k in range(2):
                    for j in range(8):
                        lg = lgA if j < 4 else lgB
                        o = (j % 4) * MC
                        ci, col = tile_map[SB * 8 + j]
                        xt = xts[ci]
                        xh_c = xt[:, k, 0, col : col + P]
                        xl_c = xt[:, k, 1, col : col + P]
                        out = lg[:, o : o + MC]
                        last = k == 1 and (j % 4) == 3
                        # xh@wh + xh@wl in ONE matmul: the out AP's
                        # stride-0 h dim folds columns 80..159 onto
                        # 0..79, accumulating both products (start=False
                        # means every column-write accumulates)
                        nc.tensor.matmul(
                            out[:, None, :].broadcast_to([P, 2, MC]),
                            lhsT=xh_c, rhs=whl_sb[:, k],
                            start=False, stop=False,
                        )
                        nc.tensor.matmul(
                            out, lhsT=xl_c, rhs=whl_sb[:, k, 0, :],
                            start=False, stop=last,
                        )

                if SB < n_sb - 1:
                    # per-group mask chains (each frees its PSUM bank);
                    # member-sum add-tree on the idle GpSimd engine
                    eqv = eqs[SB][:]
                    vote_mask(2 * SB)
                    vote_mask(2 * SB + 1)
                    t4 = tsum_pool.tile([P, 2, 4, 4, C], F16, name="t4")
                    nc.gpsimd.tensor_tensor(
                        out=t4,
                        in0=eqv[:, :, :, 0:4, :], in1=eqv[:, :, :, 4:8, :],
                        op=mybir.AluOpType.add,
                    )
                    t2 = tsum_pool.tile([P, 2, 4, 2, C], F16, name="t2")
                    nc.gpsimd.tensor_tensor(
                        out=t2,
                        in0=t4[:, :, :, 0:2, :], in1=t4[:, :, :, 2:4, :],
                        op=mybir.AluOpType.add,
                    )
                    nc.gpsimd.tensor_tensor(
                        out=stg[:, SB * 8 * C : (SB + 1) * 8 * C].rearrange(
                            "p (g t c) -> p g t c", g=2, c=C
                        ),
                        in0=t2[:, :, :, 0, :], in1=t2[:, :, :, 1, :],
                        op=mybir.AluOpType.add,
                    )
                    if SB % 2 == 1:
                        # store per SB-pair: fewer ~700ns issue ops on sync
                        nc.sync.dma_start(
                            y[:, (SB - 1) * 8 * C : (SB + 1) * 8 * C],
                            stg[:, (SB - 1) * 8 * C : (SB + 1) * 8 * C],
                        )
                else:
                    nc.sync.dma_start(
                        y[:, (SB - 1) * 8 * C : SB * 8 * C],
                        stg[:, (SB - 1) * 8 * C : SB * 8 * C],
                    )
                    # final super-batch: all-DVE per-group chains + split
                    # stores to shorten the tail
                    for g in range(2):
                        eq = vote_mask(2 * SB + g)
                        t4 = tsum_pool.tile([P, 4, 4, C], F16, name="t4f")
                        nc.vector.tensor_tensor(
                            out=t4,
                            in0=eq[:, :, 0:4, :], in1=eq[:, :, 4:8, :],
                            op=mybir.AluOpType.add,
                        )
                        t2 = tsum_pool.tile([P, 4, 2, C], F16, name="t2f")
                        nc.vector.tensor_tensor(
                            out=t2,
                            in0=t4[:, :, 0:2, :], in1=t4[:, :, 2:4, :],
                            op=mybir.AluOpType.add,
                        )
                        o = SB * 8 * C + g * 4 * C
                        nc.vector.tensor_tensor(
                            out=stg[:, o : o + 4 * C].rearrange(
                                "p (t c) -> p t c", c=C
                            ),
                            in0=t2[:, :, 0, :], in1=t2[:, :, 1, :],
                            op=mybir.AluOpType.add,
                        )
                        nc.sync.dma_start(
                            y[:, o : o + 4 * C], stg[:, o : o + 4 * C]
                        )
    nc.compile()
    return nc


_NC_CACHE: dict[int, bass.Bass] = {}


def _get_nc(b_shard: int) -> bass.Bass:
    if b_shard not in _NC_CACHE:
        _NC_CACHE[b_shard] = build_nc(b_shard)
    return _NC_CACHE[b_shard]


def make_in_maps(x: np.ndarray, W: np.ndarray, b: np.ndarray):
    """Host-side prep: exact fp16 pair decomposition + per-core packing."""
    xf = np.asarray(x, dtype=np.float32)
    xh = xf.astype(np.float16)
    xl = (xf - xh.astype(np.float32)).astype(np.float16)
    # m-major columns: col index = 10*m + c; wh|wl concatenated per row
    wf = (
        np.asarray(W, dtype=np.float32).transpose(1, 0, 2).reshape(D, MC)
    )
    whf = wf.astype(np.float16)
    wlf = (wf - whf.astype(np.float32)).astype(np.float16)
    whlf = np.ascontiguousarray(np.concatenate([whf, wlf], axis=1))
    bv = np.asarray(b, dtype=np.float32).reshape(MC)  # bv[10m+c] = b[m,c]
    bh = bv.astype(np.float16)
    bl = (bv - bh.astype(np.float32)).astype(np.float16)
    bc2 = np.ascontiguousarray(
        np.stack([np.tile(bh, 4), np.tile(bl, 4)], axis=0)
    ).astype(np.float16)

    xins = np.empty((N_CORES, P, 4 * B_SHARD), dtype=np.float16)
    halves = (xh, xl)
    for i in range(N_CORES):
        r0 = i * B_SHARD
        base = 0
        for L in CHUNKS:
            for k in range(2):
                for h in range(2):
                    c0 = 4 * base + (k * 2 + h) * L
                    xins[i, :, c0 : c0 + L] = halves[h][
                        r0 + base : r0 + base + L, k * P : (k + 1) * P
                    ].T
            base += L
    return [
        {"xin": xins[i], "whl": whlf, "bc2": bc2} for i in range(N_CORES)
    ]


def _postprocess(y_raw: np.ndarray) -> np.ndarray:
    # [p, (sb g t) * 10] fp16 -> [tile*128, 10] f32 (small ints: exact)
    n_tiles = y_raw.shape[1] // C
    return (
        y_raw.reshape(P, n_tiles, C)
        .transpose(1, 0, 2)
        .reshape(n_tiles * P, C)
        .astype(np.float32)
    )


def kernel(x: np.ndarray, W: np.ndarray, b: np.ndarray, **_) -> np.ndarray:
    from concourse.bass_utils import run_bass_kernel_spmd

    assert x.shape == (B_FULL, D), x.shape
    in_maps = make_in_maps(x, W, b)
    nc = _get_nc(B_SHARD)
    res = run_bass_kernel_spmd(nc, in_maps, core_ids=list(range(N_CORES)))
    return np.concatenate(
        [_postprocess(res.results[i]["y"]) for i in range(N_CORES)], axis=0
    )
